# revision 39
# baseline (speedup 1.0000x reference)
"""Trainium2 Bass kernel for nn_DecoderLayer_90967407329666.

Decoder layer: LN1 -> QKV (+type emb) -> multi-axis RoPE -> causal SDPA
-> residual -> LN2 -> SwiGLU FFN -> residual.  B=2, T=2048, D=768, H=8,
DFF=2048, NTYPE=16, NAX=2 rotary axes of 32 dims each.

Sharding (8 cores):
  Phase 1 (token-parallel): each core owns 512 tokens (core c: batch c//4,
    tokens 512*(c%4)...) and computes LN1 + q,k (feature-major) + v
    (token-major) + type-emb + RoPE for those tokens, all 8 heads.
  TWO pipelined AllToAlls: qk slab (192-row block per head: q 0:64 rope |
    q tail 64:96 | k rope 96:160 | k tail 160:192) fires while v is still
    being computed; v slab (96-row token-major block per head) follows.
  Phase 2 (head-parallel): core c runs full causal attention for head c,
    both batches; exp trimmed to the causal column range on diagonal
    blocks; softmax sums via an appended ones-column on v.
  AllToAll #3: o goes back token-parallel (bf16).
  Phase 3 (token-parallel): residual + LN2 + SwiGLU FFN + residual.

Type-emb one-hots, the LN beta row and the v bias are folded into a 4th
DoubleRow contraction pair of the qkv matmul (aug_x), so each qk slice is
4 fp8 matmuls and LN-apply is a single mult per chunk.  The PE HAM clock
gate (cold 1.2 GHz until a full 3.4us busy window) is defeated with
warm-up matmul bursts gated on each collective's output, so every phase
starts at 2.4 GHz.  W_fc1/W_fc2 are prefetched at kernel start.
"""

import sys

sys.path.insert(0, "/opt/trn_rl_repo")

import numpy as np

import contextlib

import concourse.bacc as bacc
import concourse.bass as bass
import concourse.tile as tile
from concourse import mybir
from concourse.bass_utils import run_bass_kernel_spmd

# ---- problem constants (hardcoded per contest rules) ----
B, T = 2, 2048
D, H, DFF, NTYPE = 768, 8, 2048, 16
NAX = 2
HD = D // H            # 96
DR = HD // (NAX + 1)   # 32
EPS = 1e-5
THETA = 10000.0
N_CORES = 8
TPC = 512              # tokens per core
NSUP = 4               # supertiles per batch (2048/512)
KD = D // 128          # 6 contraction chunks over D
SCALE = 1.0 / np.sqrt(np.float32(HD))
QKB = 192              # qk slab rows per head block (q96 + k96)
VB = 96                # v slab rows per head block
BSC = 8.0              # scale for the LN beta row in the fp8 aug pair

F32 = mybir.dt.float32
F32R = mybir.dt.float32r
BF = mybir.dt.bfloat16
F8 = mybir.dt.float8e4
I32 = mybir.dt.int32
NPBF = mybir.dt.np(BF)
NPF8 = mybir.dt.np(F8)
SCL = 64.0          # fp8 weight scale (values ~0.02 are subnormal in e4m3)
ISCL = 1.0 / SCL
DR_MODE = mybir.MatmulPerfMode.DoubleRow

# Cody-Waite split of 2*pi (C1 has 12 mantissa bits -> n*C1 exact for n<2^11)
C1 = float(np.float32(np.floor(2 * np.pi * 2**9) / 2**9))
C2 = float(np.float32(2 * np.pi - C1))
C3 = float(np.float32(2 * np.pi - C1 - float(np.float32(2 * np.pi - C1))))
HALF_PI = float(np.pi / 2)

# qk output-feature permutation: 12 slices of 128 rows
#   slices 0..7  : [q_h dims 0:64 | k_h dims 0:64]   (rope rows)
#   slice  8, 9  : q tails (dims 64:96) of heads 0..3 / 4..7
#   slice 10,11  : k tails of heads 0..3 / 4..7
def _qk_colperm():
    cols = []
    for h in range(H):
        cols += list(range(96 * h, 96 * h + 64))          # q_h 0:64
        cols += list(range(768 + 96 * h, 768 + 96 * h + 64))  # k_h 0:64
    for h in range(H):
        cols += list(range(96 * h + 64, 96 * h + 96))     # q tails
    for h in range(H):
        cols += list(range(768 + 96 * h + 64, 768 + 96 * h + 96))  # k tails
    return np.array(cols)

QK_PERM = _qk_colperm()

_prog_cache = {}


def build_program():
    key = ("v2",)
    if key in _prog_cache:
        return _prog_cache[key]
    nc = bacc.Bacc("TRN2", target_bir_lowering=False, debug=False,
                   num_devices=N_CORES)
    alu = mybir.AluOpType
    act = mybir.ActivationFunctionType

    # ---------------- DRAM I/O ----------------
    xT_d = nc.dram_tensor("xT", [D, TPC], F32R, kind="ExternalInput")
    wqk_d = nc.dram_tensor("Wqk", [128, 4 * 2 * 1536], F8,
                           kind="ExternalInput")
    wv_d = nc.dram_tensor("Wv", [128, 4 * 2 * D], F8, kind="ExternalInput")
    qtype_d = nc.dram_tensor("qtype", [1, TPC], F32R, kind="ExternalInput")
    ktype_d = nc.dram_tensor("ktype", [1, TPC], F32R, kind="ExternalInput")
    pos4_d = nc.dram_tensor("pos4", [4, TPC], F32R, kind="ExternalInput")
    invf_d = nc.dram_tensor("invf", [128, 1], F32, kind="ExternalInput")
    w1_d = nc.dram_tensor("W1", [128, 3 * 2 * 2 * DFF], F8,
                          kind="ExternalInput")
    w2_d = nc.dram_tensor("W2", [128, 8 * 2 * D], F8, kind="ExternalInput")
    b1a_d = nc.dram_tensor("b1a", [128, 16], F32, kind="ExternalInput")
    b1g_d = nc.dram_tensor("b1g", [128, 16], F32, kind="ExternalInput")
    bf2_d = nc.dram_tensor("bf2", [128, KD], F32, kind="ExternalInput")
    tri_d = nc.dram_tensor("tri", [128, 128], BF, kind="ExternalInput")
    r128_d = nc.dram_tensor("R128", [128, 128], BF, kind="ExternalInput")
    b4_d = nc.dram_tensor("B4", [4, 128], F32R, kind="ExternalInput")
    ones_d = nc.dram_tensor("ones128", [1, 128], F32R, kind="ExternalInput")
    onescol_d = nc.dram_tensor("onescol", [128, 1], F32R, kind="ExternalInput")
    onescolbf_d = nc.dram_tensor("onescolbf", [128, 1], BF,
                                 kind="ExternalInput")
    iota_d = nc.dram_tensor("iota16", [16, 1], F32, kind="ExternalInput")
    eps_d = nc.dram_tensor("epsc", [1, 1], F32, kind="ExternalInput")
    outT_d = nc.dram_tensor("outT", [D, TPC], F32, kind="ExternalOutput")

    with tile.TileContext(nc) as tc:
        with tc.tile_pool(name="glob", bufs=1) as glob, \
             tc.tile_pool(name="dram", bufs=1, space="DRAM") as dram:
            # exchange slabs (bf16)
            slabqk_in = dram.tile([8 * QKB, TPC], BF, tag="slabqk_in")
            slabqk_out = dram.tile([8 * QKB, TPC], BF, tag="slabqk_out")
            slabv_in = dram.tile([8 * VB, TPC], BF, tag="slabv_in")
            slabv_out = dram.tile([8 * VB, TPC], BF, tag="slabv_out")
            slab2_in = dram.tile([D, TPC], BF, tag="slab2_in")
            slab2_out = dram.tile([D, TPC], BF, tag="slab2_out")

            # ---- persistent constants / activations ----
            ones_sb = glob.tile([1, 128], F32R, tag="ones")
            nc.sync.dma_start(out=ones_sb[:], in_=ones_d[:])
            onescol_sb = glob.tile([128, 1], F32R, tag="onescol")
            nc.sync.dma_start(out=onescol_sb[:], in_=onescol_d[:])
            onescol_bf = glob.tile([128, 1], BF, tag="onescolbf")
            nc.sync.dma_start(out=onescol_bf[:], in_=onescolbf_d[:])
            tri_sb = glob.tile([128, 128], BF, tag="tri")
            nc.sync.dma_start(out=tri_sb[:], in_=tri_d[:])
            xT = []
            for k in range(KD):
                t = glob.tile([128, TPC], F32R, tag=f"xT{k}")
                nc.sync.dma_start(out=t[:], in_=xT_d[128 * k:128 * (k + 1), :])
                xT.append(t)
            iota_sb = glob.tile([16, 1], F32, tag="iota")
            nc.sync.dma_start(out=iota_sb[:], in_=iota_d[:])
            eps_sb = glob.tile([1, 1], F32, tag="eps")
            nc.sync.dma_start(out=eps_sb[:], in_=eps_d[:])
            b1a_sb = glob.tile([128, 16], F32, tag="b1a")
            nc.sync.dma_start(out=b1a_sb[:], in_=b1a_d[:])
            b1g_sb = glob.tile([128, 16], F32, tag="b1g")
            nc.sync.dma_start(out=b1g_sb[:], in_=b1g_d[:])
            bf2_sb = glob.tile([128, KD], F32, tag="bf2")
            nc.sync.dma_start(out=bf2_sb[:], in_=bf2_d[:])

            def warm_burst(pool_sb, lhsT, rhs, n, tag):
                """Gapless matmul burst to flip the PE HAM gate to 2.4 GHz.

                lhsT/rhs must depend on the data that gates the next phase
                so the burst runs right when the wait ends, not earlier.
                """
                with tc.tile_pool(name=f"wm{tag}", bufs=1,
                                  space="PSUM") as wp:
                    ps = wp.tile([128, 128], F32, tag=f"wm{tag}")
                    for i in range(n):
                        nc.tensor.matmul(ps[:], lhsT, rhs,
                                         start=(i == 0), stop=(i == n - 1))
                    snk = pool_sb.tile([1, 128], F32, tag=f"wmsnk{tag}")
                    nc.vector.tensor_copy(out=snk[:], in_=ps[0:1, :])

            def layernorm_stats(pool, persist, src_tiles, tag):
                """src (fp32r views, 6 x (128,TPC)) -> (alpha, beta) rows."""
                ctx = contextlib.ExitStack()
                ps_pool = ctx.enter_context(
                    tc.tile_pool(name=f"{tag}ps", bufs=1, space="PSUM"))
                sums = ps_pool.tile([1, TPC], F32, tag=f"{tag}sums")
                sumsq = ps_pool.tile([1, TPC], F32, tag=f"{tag}sumsq")
                for k in range(KD):
                    # square on ACT so DVE stays free for the residual adds
                    sq = pool.tile([128, TPC], F32R, tag="lnsq", bufs=2)
                    nc.scalar.activation(out=sq[:],
                                         in_=src_tiles[k].bitcast(F32),
                                         func=act.Square)
                    nc.tensor.matmul(sums[:], onescol_sb[:], src_tiles[k],
                                     start=(k == 0), stop=(k == KD - 1))
                    nc.tensor.matmul(sumsq[:], onescol_sb[:], sq[:],
                                     start=(k == 0), stop=(k == KD - 1))
                mean = pool.tile([1, TPC], F32, tag="lnmean")
                nc.vector.tensor_scalar(out=mean[:], in0=sums[:],
                                        scalar1=1.0 / D, scalar2=None,
                                        op0=alu.mult)
                var = pool.tile([1, TPC], F32, tag="lnvar")
                nc.vector.tensor_tensor(out=var[:], in0=mean[:], in1=mean[:],
                                        op=alu.mult)
                nc.vector.scalar_tensor_tensor(
                    out=var[:], in0=sumsq[:], scalar=1.0 / D, in1=var[:],
                    op0=alu.mult, op1=alu.subtract)
                std = pool.tile([1, TPC], F32, tag="lnstd")
                nc.scalar.activation(out=std[:], in_=var[:], func=act.Sqrt,
                                     bias=eps_sb[:])
                rstd = pool.tile([1, TPC], F32, tag="lnrstd")
                nc.vector.reciprocal_approx_fast(out=rstd[:], in_=std[:])
                alpha = persist.tile([1, TPC], F32R, tag=f"{tag}alpha")
                with nc.allow_low_precision(reason="fp32r bcast rhs"):
                    nc.vector.tensor_copy(out=alpha[:], in_=rstd[:])
                beta = persist.tile([1, TPC], F32R, tag=f"{tag}beta")
                nc.vector.scalar_tensor_tensor(
                    out=beta[:], in0=mean[:], scalar=-1.0,
                    in1=alpha[:].bitcast(F32), op0=alu.mult, op1=alu.mult)
                ctx.close()
                return alpha, beta

            def ln_apply(pool, scratch, src_f32_aps, alpha, beta, tag):
                """3 fp8 tiles [128, 2, TPC] (DR pairs): xhat = src*ab (+bb).

                beta=None when the LN beta term is folded into a matmul
                aug pair downstream (LN1); else full xhat (LN2).
                """
                ctx = contextlib.ExitStack()
                ps_pool = ctx.enter_context(
                    tc.tile_pool(name=f"{tag}ps", bufs=1, space="PSUM"))
                ab = ps_pool.tile([128, TPC], F32, tag=f"{tag}ab")
                nc.tensor.matmul(ab[:], ones_sb[:], alpha[:], start=True,
                                 stop=True)
                if beta is not None:
                    bb = ps_pool.tile([128, TPC], F32, tag=f"{tag}bb")
                    nc.tensor.matmul(bb[:], ones_sb[:], beta[:], start=True,
                                     stop=True)
                xps = []
                for kp in range(KD // 2):
                    xp = pool.tile([128, 2, TPC], F8, tag=f"{tag}xp{kp}")
                    for i in range(2):
                        k = 2 * kp + i
                        if beta is None:
                            nc.vector.tensor_tensor(out=xp[:, i, :],
                                                    in0=src_f32_aps[k],
                                                    in1=ab[:], op=alu.mult)
                        else:
                            t1 = scratch.tile([128, TPC], F32, tag="lnt1",
                                              bufs=2)
                            nc.vector.tensor_tensor(out=t1[:],
                                                    in0=src_f32_aps[k],
                                                    in1=ab[:], op=alu.mult)
                            nc.vector.tensor_tensor(out=xp[:, i, :],
                                                    in0=t1[:], in1=bb[:],
                                                    op=alu.add)
                    xps.append(xp)
                ctx.close()
                return xps

            # ================= PHASE 1 =================
            with contextlib.ExitStack() as p1:
                p1t = p1.enter_context(tc.tile_pool(name="p1t", bufs=1))
                p1w = p1.enter_context(tc.tile_pool(name="p1w", bufs=1))

                # phase-1 weights (freed at the AllToAll), DoubleRow-packed
                wqkdr = p1w.tile([128, 4, 2, 1536], F8, tag="wqkdr")
                nc.sync.dma_start(out=wqkdr[:], in_=wqk_d[:])
                wvdr = p1w.tile([128, 4, 2, D], F8, tag="wvdr")
                nc.sync.dma_start(out=wvdr[:], in_=wv_d[:])
                r128_sb = p1w.tile([128, 128], BF, tag="r128")
                nc.sync.dma_start(out=r128_sb[:], in_=r128_d[:])
                b4_sb = p1w.tile([4, 128], F32R, tag="b4")
                nc.sync.dma_start(out=b4_sb[:], in_=b4_d[:])
                invf_sb = p1w.tile([128, 1], F32, tag="invf")
                nc.sync.dma_start(out=invf_sb[:], in_=invf_d[:])
                pos4_sb = p1w.tile([4, TPC], F32R, tag="pos4")
                nc.sync.dma_start(out=pos4_sb[:], in_=pos4_d[:])
                qt_sb = p1w.tile([1, TPC], F32R, tag="qt")
                nc.sync.dma_start(out=qt_sb[:], in_=qtype_d[:])
                kt_sb = p1w.tile([1, TPC], F32R, tag="kt")
                nc.sync.dma_start(out=kt_sb[:], in_=ktype_d[:])
                # FFN weights prefetched now (after phase-1 loads in the
                # sync queue): traffic hides under phase 1 + AllToAll
                w1dr = glob.tile([128, 3, 2, 2 * DFF], F8, tag="w1dr")
                nc.sync.dma_start(out=w1dr[:], in_=w1_d[:])
                w2dr = glob.tile([128, 8, 2, D], F8, tag="w2dr")
                nc.sync.dma_start(out=w2dr[:], in_=w2_d[:])

                # warm the PE while xT streams in (tri lands first)
                warm_burst(p1t, tri_sb[:], tri_sb[:], 56, "p1")

                # LN1
                a1, be1 = layernorm_stats(p1t, p1w, [x[:] for x in xT], "l1")
                xn = ln_apply(p1w, p1t, [x[:].bitcast(F32) for x in xT],
                              a1, None, "l1a")

                # aug pair: one-hot type rows + LN beta row, consumed as
                # the 4th DoubleRow contraction pair
                aug_x = p1w.tile([128, 2, TPC], F8, tag="augx")
                nc.vector.memset(aug_x[:], 0.0)
                nc.vector.tensor_scalar(out=aug_x[32:33, 0, :],
                                        in0=be1[:].bitcast(F32),
                                        scalar1=float(BSC), scalar2=None,
                                        op0=alu.mult)
                with contextlib.ExitStack() as oh_ctx:
                    p1oh = oh_ctx.enter_context(
                        tc.tile_pool(name="p1oh", bufs=1, space="PSUM"))

                    def onehot(row_sb, slot):
                        bc = p1oh.tile([16, TPC], F32, tag="ohbc", bufs=2)
                        nc.tensor.matmul(bc[:], ones_sb[:, 0:16], row_sb[:],
                                         start=True, stop=True)
                        nc.vector.tensor_scalar(out=aug_x[0:16, slot, :],
                                                in0=bc[:],
                                                scalar1=iota_sb[:],
                                                scalar2=None,
                                                op0=alu.is_equal)
                    onehot(qt_sb, 0)
                    onehot(kt_sb, 1)

                with contextlib.ExitStack() as pm_ctx:
                    p1misc = pm_ctx.enter_context(
                        tc.tile_pool(name="p1misc", bufs=1, space="PSUM"))
                    # cos/sin tiles (128, TPC): rows 0:64 q-axes, 64:128 k
                    pm = p1misc.tile([128, TPC], F32, tag="pm")
                    nc.tensor.matmul(pm[:], b4_sb[:], pos4_sb[:], start=True,
                                     stop=True)
                    f_t = p1t.tile([128, TPC], F32, tag="f")
                    nc.vector.tensor_scalar(out=f_t[:], in0=pm[:],
                                            scalar1=invf_sb[:], scalar2=None,
                                            op0=alu.mult)
                nt = p1t.tile([128, TPC], F32, tag="nt")
                nc.vector.tensor_scalar(out=nt[:], in0=f_t[:],
                                        scalar1=float(1.0 / (2 * np.pi)),
                                        scalar2=None, op0=alu.mult)
                n_i = p1t.tile([128, TPC], I32, tag="ni")
                nc.vector.tensor_copy(out=n_i[:], in_=nt[:])
                nc.vector.tensor_copy(out=nt[:], in_=n_i[:])
                # Cody-Waite range reduction, single custom-DVE op
                nc.vector.cody_waite_cascade(out=f_t[:], x=f_t[:], k=nt[:],
                                             c1=C1, c2=C2, c3=C3)
                s_t = p1t.tile([128, TPC], F32, tag="sin")
                nc.scalar.activation(out=s_t[:], in_=f_t[:], func=act.Sin)
                nc.scalar.activation(out=nt[:], in_=f_t[:], func=act.Abs)
                nc.vector.tensor_scalar(out=nt[:], in0=nt[:], scalar1=-1.0,
                                        scalar2=HALF_PI, op0=alu.mult,
                                        op1=alu.add)
                c_t = p1t.tile([128, TPC], F32, tag="cos")
                nc.scalar.activation(out=c_t[:], in_=nt[:], func=act.Sin)
                # fold score scale 1/sqrt(HD) into q: scale c,s rows 0:64
                nc.vector.tensor_scalar(out=c_t[0:64, :], in0=c_t[0:64, :],
                                        scalar1=float(SCALE), scalar2=None,
                                        op0=alu.mult)
                nc.vector.tensor_scalar(out=s_t[0:64, :], in0=s_t[0:64, :],
                                        scalar1=float(SCALE), scalar2=None,
                                        op0=alu.mult)
                c_b = p1w.tile([128, TPC], BF, tag="cosb")
                nc.vector.tensor_copy(out=c_b[:], in_=c_t[:])
                s_b = p1w.tile([128, TPC], BF, tag="sinb")
                nc.vector.tensor_copy(out=s_b[:], in_=s_t[:])

                # qk slices: 4 DR matmuls each, then rope / tails -> slab
                p1qk = p1.enter_context(
                    tc.tile_pool(name="p1qk", bufs=1, space="PSUM"))
                for s in range(12):
                    qk_ps = p1qk.tile([128, TPC], F32, tag="qkps", bufs=4)
                    for kp in range(3):
                        nc.tensor.matmul(qk_ps[:],
                                         wqkdr[:, kp, :,
                                               128 * s:128 * (s + 1)],
                                         xn[kp][:],
                                         start=(kp == 0), stop=False,
                                         perf_mode=DR_MODE)
                    nc.tensor.matmul(qk_ps[:],
                                     wqkdr[:, 3, :, 128 * s:128 * (s + 1)],
                                     aug_x[:], start=False, stop=True,
                                     perf_mode=DR_MODE)
                    if s < 8:
                        # rope: q_h 0:64 | k_h 0:64 (1/SCL descale)
                        rsb = p1t.tile([128, TPC], BF, tag="rsb", bufs=2)
                        nc.scalar.activation(out=rsb[:], in_=qk_ps[:],
                                             func=act.Copy, scale=ISCL)
                        rot = p1qk.tile([128, TPC], F32, tag="rot", bufs=2)
                        nc.tensor.matmul(rot[:], r128_sb[:], rsb[:],
                                         start=True, stop=True)
                        t1 = p1t.tile([128, TPC], BF, tag="rt1", bufs=2)
                        nc.gpsimd.tensor_tensor(out=t1[:], in0=rsb[:],
                                                in1=c_b[:], op=alu.mult)
                        t2 = p1t.tile([128, TPC], BF, tag="rt2", bufs=2)
                        nc.vector.tensor_tensor(out=t2[:], in0=rot[:],
                                                in1=s_b[:], op=alu.mult)
                        qkr = p1t.tile([128, TPC], BF, tag="qkr", bufs=2)
                        nc.vector.tensor_tensor(out=qkr[:], in0=t1[:],
                                                in1=t2[:], op=alu.add)
                        # single DMA: q rows 0:64 -> block base, k rows
                        # 64:128 -> block base+96
                        h = s
                        dst = bass.AP(
                            tensor=slabqk_in[:].tensor,
                            offset=QKB * h * TPC,
                            ap=[[96 * TPC, 2], [TPC, 64], [1, TPC]])
                        eng = nc.scalar if s % 2 else nc.sync
                        eng.dma_start(out=dst, in_=qkr[:])
                    else:
                        # tails: s=8,9 q tails h0..3/h4..7 (scaled); 10,11 k
                        tl = p1t.tile([128, TPC], BF, tag="tail", bufs=2)
                        sc = float(SCALE) * ISCL if s < 10 else ISCL
                        nc.vector.tensor_scalar(out=tl[:], in0=qk_ps[:],
                                                scalar1=sc, scalar2=None,
                                                op0=alu.mult)
                        base = 64 if s < 10 else 160
                        h0 = 4 * (s % 2)
                        dst = bass.AP(
                            tensor=slabqk_in[:].tensor,
                            offset=(QKB * h0 + base) * TPC,
                            ap=[[QKB * TPC, 4], [TPC, 32], [1, TPC]])
                        eng = nc.scalar if s % 2 else nc.sync
                        eng.dma_start(out=dst, in_=tl[:])

                nc.gpsimd.collective_compute(
                    "AllToAll", mybir.AluOpType.bypass,
                    replica_groups=[list(range(N_CORES))],
                    ins=[slabqk_in[:].opt()],
                    outs=[slabqk_out[:].opt()])

                # v (token-major): computed while the qk exchange flies
                for ts_ in range(4):
                    for hf in range(2):
                        v_ps = p1qk.tile([128, 384], F32, tag="vps", bufs=2)
                        for kp in range(3):
                            nc.tensor.matmul(
                                v_ps[:],
                                xn[kp][:, :, 128 * ts_:128 * (ts_ + 1)],
                                wvdr[:, kp, :, 384 * hf:384 * (hf + 1)],
                                start=(kp == 0), stop=False,
                                perf_mode=DR_MODE)
                        nc.tensor.matmul(
                            v_ps[:],
                            aug_x[:, :, 128 * ts_:128 * (ts_ + 1)],
                            wvdr[:, 3, :, 384 * hf:384 * (hf + 1)],
                            start=False, stop=True, perf_mode=DR_MODE)
                        v_sb1 = p1t.tile([128, 384], BF, tag="vsb1", bufs=2)
                        nc.vector.tensor_scalar(out=v_sb1[:], in0=v_ps[:],
                                                scalar1=ISCL, scalar2=None,
                                                op0=alu.mult)
                        # one DMA for 4 heads: [tok 128][head 4][feat 96]
                        dst = bass.AP(
                            tensor=slabv_in[:].tensor,
                            offset=VB * 4 * hf * TPC + 128 * ts_ * 96,
                            ap=[[96, 128], [VB * TPC, 4], [1, 96]])
                        eng = nc.scalar if (ts_ + hf) % 2 else nc.sync
                        eng.dma_start(out=dst, in_=v_sb1[:])

                nc.gpsimd.collective_compute(
                    "AllToAll", mybir.AluOpType.bypass,
                    replica_groups=[list(range(N_CORES))],
                    ins=[slabv_in[:].opt()],
                    outs=[slabv_out[:].opt()])

            # ================= PHASE 2 =================
            with contextlib.ExitStack() as p2:
                p2w = p2.enter_context(tc.tile_pool(name="p2w", bufs=1))

                qTs, kTs, vs = [], [], []
                for bb_ in range(2):
                    qT = p2w.tile([96, 2048], BF, tag=f"qT{bb_}")
                    kT = p2w.tile([96, 2048], BF, tag=f"kT{bb_}")
                    v_sb = p2w.tile([128, 16, 97], BF, tag=f"v{bb_}")
                    # unpack spread across engine DMA queues (the ~1us
                    # per-DMA setup serializes within one queue)
                    eng_a = nc.sync if bb_ == 0 else nc.scalar
                    eng_b = nc.gpsimd
                    src_q = bass.AP(
                        tensor=slabqk_out[:].tensor,
                        offset=QKB * 4 * bb_ * TPC,
                        ap=[[TPC, 96], [QKB * TPC, 4], [1, TPC]])
                    eng_a.dma_start(out=qT[:], in_=src_q)
                    src_k = bass.AP(
                        tensor=slabqk_out[:].tensor,
                        offset=(QKB * 4 * bb_ + 96) * TPC,
                        ap=[[TPC, 96], [QKB * TPC, 4], [1, TPC]])
                    eng_a.dma_start(out=kT[:], in_=src_k)
                    ones_bc = bass.AP(
                        tensor=onescol_bf[:].tensor,
                        offset=onescol_bf[:].offset,
                        ap=[[1, 128], [0, 16], [0, 1]])
                    eng_b.dma_start(out=v_sb[:, :, 0:1], in_=ones_bc)
                    for u in range(4):
                        src_v = bass.AP(
                            tensor=slabv_out[:].tensor,
                            offset=VB * (4 * bb_ + u) * TPC,
                            ap=[[96, 128], [128 * 96, 4], [1, 96]])
                        eng_b.dma_start(out=v_sb[:, 4 * u:4 * u + 4, 1:97],
                                        in_=src_v)
                    qTs.append(qT); kTs.append(kT); vs.append(v_sb)

                # re-warm the PE during the tail of the v exchange
                warm_burst(p2w, qTs[0][:, 0:128], qTs[0][:, 0:128], 40, "p2")

                p2t = p2.enter_context(tc.tile_pool(name="p2t", bufs=3))
                p2ps = p2.enter_context(
                    tc.tile_pool(name="p2ps", bufs=4, space="PSUM"))
                p2o = p2.enter_context(
                    tc.tile_pool(name="p2o", bufs=2, space="PSUM"))
                p2rb = p2.enter_context(
                    tc.tile_pool(name="p2rb", bufs=1, space="PSUM"))

                # batches interleaved: while one batch's unit waits on
                # exp/normalization, the other's s-matmuls keep PE busy
                for Q in reversed(range(NSUP)):
                    for bb_ in range(2):
                        qT, kT, v_sb = qTs[bb_], kTs[bb_], vs[bb_]
                        o_ps = p2o.tile([97, 512], F32, tag="ops",
                                        name="ops")
                        nkt = 4 * Q + 4
                        for kt in range(nkt):
                            s_ps = p2ps.tile([128, 512], F32, tag="sps",
                                             name="sps")
                            nc.tensor.matmul(
                                s_ps[:], kT[:, 128 * kt:128 * (kt + 1)],
                                qT[:, 512 * Q:512 * (Q + 1)],
                                start=True, stop=True)
                            e_sb = p2t.tile([128, 512], BF, tag="esb",
                                            name="esb", bufs=14)
                            dj = kt - 4 * Q
                            if dj > 0:
                                # cols < 128*dj fully masked: zero, skip exp
                                nc.vector.memset(e_sb[:, 0:128 * dj], 0.0)
                            lo = 128 * dj if dj > 0 else 0
                            nc.scalar.activation(out=e_sb[:, lo:],
                                                 in_=s_ps[:, lo:],
                                                 func=act.Exp)
                            if dj >= 0:
                                # triangular strip: one 128-col mask mult
                                nc.vector.tensor_tensor(
                                    out=e_sb[:, lo:lo + 128],
                                    in0=e_sb[:, lo:lo + 128],
                                    in1=tri_sb[:], op=alu.mult)
                            nc.tensor.matmul(o_ps[:], v_sb[:, kt, :],
                                             e_sb[:],
                                             start=(kt == 0),
                                             stop=(kt == nkt - 1))
                        # per-unit normalization, overlapped with the next
                        # unit's attention; den sits on PSUM partition 0
                        # (ones column is slot 0 of v_sb)
                        j = 4 * bb_ + Q
                        o_u = p2w.tile([97, 512], BF, tag="ou", bufs=2)
                        nc.vector.tensor_copy(out=o_u[:], in_=o_ps[:])
                        rdj = p2w.tile([1, 512], F32, tag="rdj", bufs=2)
                        nc.vector.reciprocal_approx_fast(
                            out=rdj[:], in_=o_ps[0:1, :])
                        rcj = p2w.tile([1, 512], F32R, tag="rcj", bufs=2)
                        with nc.allow_low_precision(reason="fp32r bcast rhs"):
                            nc.vector.tensor_copy(out=rcj[:], in_=rdj[:])
                        rb = p2rb.tile([97, 512], F32, tag="rb", bufs=2)
                        nc.tensor.matmul(rb[:], ones_sb[:, 0:97], rcj[:],
                                         start=True, stop=True)
                        onrm = p2t.tile([97, 512], BF, tag="onrm", bufs=2)
                        nc.vector.tensor_tensor(out=onrm[:], in0=o_u[:],
                                                in1=rb[:], op=alu.mult)
                        nc.scalar.dma_start(
                            out=slab2_in[96 * j:96 * (j + 1), :],
                            in_=onrm[1:97, :])

            nc.gpsimd.collective_compute(
                "AllToAll", mybir.AluOpType.bypass,
                replica_groups=[list(range(N_CORES))],
                ins=[slab2_in[:].opt()], outs=[slab2_out[:].opt()])

            # ================= PHASE 3 =================
            with contextlib.ExitStack() as p3:
                p3w = p3.enter_context(tc.tile_pool(name="p3w", bufs=1))
                p3t = p3.enter_context(tc.tile_pool(name="p3t", bufs=2))
                # o unpack: two bulk DMAs on separate queues
                o_all = p3w.tile([128, KD, TPC], BF, tag="o_all")
                for half, eng in ((0, nc.sync), (1, nc.scalar)):
                    src_o = bass.AP(
                        tensor=slab2_out[:].tensor,
                        offset=3 * half * 128 * TPC,
                        ap=[[TPC, 128], [128 * TPC, 3], [1, TPC]])
                    eng.dma_start(out=o_all[:, 3 * half:3 * half + 3, :],
                                  in_=src_o)
                # re-warm the PE while the o chunks stream in
                warm_burst(p3w, o_all[:, 0, 0:128], o_all[:, 0, 0:128],
                           40, "p3")
                x2 = []
                for k in range(KD):
                    t = p3w.tile([128, TPC], F32R, tag=f"x2_{k}")
                    nc.vector.tensor_tensor(out=t[:], in0=o_all[:, k, :],
                                            in1=xT[k][:].bitcast(F32),
                                            op=alu.add)
                    x2.append(t)

                a2, be2 = layernorm_stats(
                    p3t, p3w, [t[:] for t in x2], "l2")
                x2n = ln_apply(p3w, p3t,
                               [t[:].bitcast(F32) for t in x2], a2, be2,
                               "l2a")

                # re-warm right before fc1, gated on the first LN2
                # apply chunk so it fires just as fc1 becomes runnable
                warm_burst(p3w, x2n[0][:, 0, 0:128], x2n[0][:, 0, 0:128],
                           36, "p3b")

                # fc1: DoubleRow fp8, 3 K-pair matmuls per output tile
                a_tiles = []
                sw = []
                for kp in range(8):
                    sw_t = p3w.tile([128, 2, TPC], F8, tag=f"sw{kp}",
                                    name=f"sw{kp}")
                    sw.append(sw_t)
                with tc.tile_pool(name="p3h", bufs=3, space="PSUM") as p3h:
                    for g in range(8):           # g<4: a-half, g>=4: gate
                        for mi in range(4):
                            i = 4 * (g % 4) + mi
                            col = 512 * g + 128 * mi
                            h_ps = p3h.tile([128, TPC], F32, tag="hps")
                            for kp in range(3):
                                nc.tensor.matmul(
                                    h_ps[:],
                                    w1dr[:, kp, :, col:col + 128],
                                    x2n[kp][:],
                                    start=(kp == 0), stop=(kp == 2),
                                    perf_mode=DR_MODE)
                            if g < 4:
                                a_sb = p3w.tile([128, TPC], BF, tag=f"a{i}")
                                nc.vector.tensor_scalar(
                                    out=a_sb[:], in0=h_ps[:],
                                    scalar1=ISCL,
                                    scalar2=b1a_sb[:, i:i + 1],
                                    op0=alu.mult, op1=alu.add)
                                a_tiles.append(a_sb)
                            else:
                                sil = p3t.tile([128, TPC], BF, tag="sil")
                                nc.scalar.activation(
                                    out=sil[:], in_=h_ps[:], func=act.Silu,
                                    bias=b1g_sb[:, i:i + 1], scale=ISCL)
                                nc.vector.tensor_tensor(
                                    out=sw[i // 2][:, i % 2, :],
                                    in0=sil[:],
                                    in1=a_tiles[i][:], op=alu.mult)

                # fc2: DoubleRow fp8, d-outer so each output chunk
                # finalizes (and DMAs out) while later chunks accumulate
                with tc.tile_pool(name="p3f", bufs=2, space="PSUM") as p3f:
                    for d in range(KD):
                        ff = p3f.tile([128, TPC], F32, tag="ff", name="ff")
                        for kp in range(8):
                            nc.tensor.matmul(ff[:],
                                             w2dr[:, kp, :,
                                                  128 * d:128 * (d + 1)],
                                             sw[kp][:],
                                             start=(kp == 0), stop=(kp == 7),
                                             perf_mode=DR_MODE)
                        t = p3t.tile([128, TPC], F32, tag="fft")
                        nc.vector.tensor_scalar(out=t[:], in0=ff[:],
                                                scalar1=ISCL,
                                                scalar2=bf2_sb[:, d:d + 1],
                                                op0=alu.mult, op1=alu.add)
                        o = p3t.tile([128, TPC], F32, tag="oout")
                        nc.vector.tensor_tensor(out=o[:], in0=t[:],
                                                in1=x2[d][:].bitcast(F32),
                                                op=alu.add)
                        nc.sync.dma_start(
                            out=outT_d[128 * d:128 * (d + 1), :], in_=o[:])

    nc.compile()
    _prog_cache[key] = nc
    return nc


def _host_inputs(x_type, x_value, seq_order, W_attn, type_emb, g1, b1, g2, b2,
                 W_fc1, b_fc1, W_fc2, b_fc2):
    f32 = np.float32
    x_type = np.asarray(x_type)
    seq_order = np.asarray(seq_order)
    x_value = np.asarray(x_value, dtype=f32)
    W_attn = np.asarray(W_attn, dtype=f32)
    type_emb = np.asarray(type_emb, dtype=f32)
    W_fc1 = np.asarray(W_fc1, dtype=f32)
    W_fc2 = np.asarray(W_fc2, dtype=f32)
    g1 = np.asarray(g1, f32); b1 = np.asarray(b1, f32)
    g2 = np.asarray(g2, f32); b2 = np.asarray(b2, f32)
    b_fc1 = np.asarray(b_fc1, f32); b_fc2 = np.asarray(b_fc2, f32)

    # fold LN gains/biases into the weights:
    #   qkv = LN(x)@W = (xhat*g1 + b1)@W = xhat@(g1[:,None]*W) + b1@W
    # xhat = x*alpha + beta*1; the beta rank-1 term rides in an augmented
    # 4th DoubleRow pair (beta*BSC row on the x side, colsum(W)/BSC row on
    # the weight side), together with the type-emb one-hots and v bias.
    Wg = W_attn * g1[:, None]
    bW = b1 @ W_attn                       # (2304,)
    wqk_full = Wg[:, :1536][:, QK_PERM].copy()
    te_full = type_emb[:, QK_PERM]         # (16, 1536)
    bW_qk = bW[:1536][QK_PERM]
    q_origin = QK_PERM < 768
    te_q = np.where(q_origin[None, :], te_full + bW_qk[None, :], 0.0)
    te_k = np.where(~q_origin[None, :], te_full + bW_qk[None, :], 0.0)
    bWv = bW[1536:]                        # (768,)

    wqk_ext = np.zeros((1024, 1536), f32)
    wqk_ext[0:768] = wqk_full
    wqk_ext[768:784] = te_q                # aug slot 0, partitions 0:16
    wqk_ext[800] = wqk_full.sum(axis=0) / BSC   # beta row (slot 0, p=32)
    wqk_ext[896:912] = te_k                # aug slot 1, partitions 0:16

    wv_ext = np.zeros((1024, D), f32)
    wv_ext[0:768] = Wg[:, 1536:]
    # v bias rides the one-hot rows (sum of one-hots is 1 per token)
    wv_ext[768:784] = bWv[None, :]
    wv_ext[800] = Wg[:, 1536:].sum(axis=0) / BSC

    W1g = W_fc1 * g2[:, None]
    b_fc1_eff = b_fc1 + b2 @ W_fc1         # (4096,)

    invf16 = (1.0 / THETA ** (np.arange(0, DR, 2, dtype=f32) / DR)).astype(f32)
    invf_col = invf16[(np.arange(128) % 32) // 2].reshape(128, 1)

    # triangular 128x128 strip mask: tri[kk, c] = 1 if c >= kk
    kk = np.arange(128)[:, None]
    cc = np.arange(128)[None, :]
    tri = (cc >= kk).astype(NPBF)

    # rot lhsT: lhsT[k, m] = P[m, k];  P[2i, 2i+1] = -1, P[2i+1, 2i] = +1
    R = np.zeros((128, 128), f32)
    for i in range(64):
        R[2 * i + 1, 2 * i] = -1.0
        R[2 * i, 2 * i + 1] = 1.0
    B4m = np.zeros((4, 128), f32)
    B4m[0, 0:32] = 1.0; B4m[1, 32:64] = 1.0
    B4m[2, 64:96] = 1.0; B4m[3, 96:128] = 1.0

    def pack_dr(W):
        # (256*nkp, cols) -> (128, nkp*2*cols) DoubleRow pair layout
        K, cols = W.shape
        nkp = K // 256
        out = np.empty((128, nkp, 2, cols), np.float32)
        for kp in range(nkp):
            for i in range(2):
                out[:, kp, i, :] = W[256 * kp + 128 * i:
                                     256 * kp + 128 * i + 128, :]
        return np.ascontiguousarray(out.reshape(128, nkp * 2 * cols))

    common = {
        "Wqk": pack_dr(wqk_ext * SCL).astype(NPF8),
        "Wv": pack_dr(wv_ext * SCL).astype(NPF8),
        "invf": invf_col,
        "W1": pack_dr(W1g * SCL).astype(NPF8),
        "W2": pack_dr(W_fc2 * SCL).astype(NPF8),
        "b1a": b_fc1_eff[:2048].reshape(16, 128).T.copy(),
        "b1g": b_fc1_eff[2048:].reshape(16, 128).T.copy(),
        "bf2": b_fc2.reshape(6, 128).T.copy(),
        "tri": tri, "R128": R.astype(NPBF), "B4": B4m,
        "ones128": np.ones((1, 128), f32),
        "onescol": np.ones((128, 1), f32),
        "onescolbf": np.ones((128, 1), NPBF),
        "iota16": np.arange(16, dtype=f32).reshape(16, 1),
        "epsc": np.full((1, 1), EPS, f32),
    }
    in_maps = []
    for c in range(N_CORES):
        b = c // 4
        t0 = 512 * (c % 4)
        m = dict(common)
        m["xT"] = np.ascontiguousarray(x_value[b, t0:t0 + TPC, :].T)
        m["qtype"] = x_type[b, t0:t0 + TPC].astype(f32).reshape(1, TPC)
        m["ktype"] = x_type[b, t0 + 1:t0 + TPC + 1].astype(f32).reshape(1, TPC)
        pos4 = np.stack([
            seq_order[0, b, t0:t0 + TPC],
            seq_order[1, b, t0:t0 + TPC],
            seq_order[0, b, t0 + 1:t0 + TPC + 1],
            seq_order[1, b, t0 + 1:t0 + TPC + 1],
        ]).astype(f32)
        m["pos4"] = pos4
        in_maps.append(m)
    return in_maps


def kernel(**inputs):
    nc = build_program()
    in_maps = _host_inputs(**inputs)
    res = run_bass_kernel_spmd(nc, in_maps, list(range(N_CORES)), trace=False)
    out = np.empty((B, T, D), np.float32)
    for c in range(N_CORES):
        b = c // 4
        t0 = 512 * (c % 4)
        out[b, t0:t0 + TPC, :] = res.results[c]["outT"].T
    return out

